# revision 9
# baseline (speedup 1.0000x reference)
"""Trainium2 Bass kernel: Neural SDE (Euler-Maruyama scan, drift+diffusion MLPs).

Strategy:
  - Data-parallel across 8 NeuronCores over the trajectory batch (256 -> 32/core).
  - Inside each core: 2048 sequential timesteps in a For_i dynamic loop
    (U steps unrolled per iteration).
  - Activations kept feature-major [feat, batch] on SBUF partitions; all
    matmuls are weight-stationary: out[f_out, b] = (W^T)tile.T @ H.
  - Brownian increments are precomputed on host with JAX CPU, bit-matching
    the reference PRNG key chain, and streamed in via DMA.
  - The scalar time input t is folded into a per-step bias table for the
    first layer of both MLPs (z = [t, y] -> W[:,0]*t + b becomes bias(t)).
  - lipswish(x) = 0.909*silu(x): the 0.909 is folded into the next layer's
    weights; the HW Silu activation is used directly.
  - The diffusion einsum sum_n sig[b,h,n]*bm[b,n] runs on the vector engine
    with the noise weight rows permuted so each 128-row tile has a fixed n.
"""

import sys

if "/opt/trn_rl_repo" not in sys.path:
    sys.path.insert(0, "/opt/trn_rl_repo")

import numpy as np

B, HIDDEN, NOISE, WIDTH = 256, 128, 32, 256
NHID = 3
T = 2048
NCORES = 8
BS = B // NCORES  # 32 per-core batch
U = 8             # timesteps per For_i iteration

# weight/hidden-activation dtype: "f32", "f16" (half) or "bf16"
DTYPE_W = "f32"

_CACHE = {}


def _gen_bm(seed, num_t, dt):
    """Replicate reference's jax.random key chain for Brownian increments."""
    import jax
    import jax.numpy as jnp

    cpu = jax.devices("cpu")[0]
    with jax.default_device(cpu):
        key = jax.random.key(int(seed))
        _, bm_key = jax.random.split(key)
        sqrt_dt = jnp.sqrt(jnp.float32(dt))

        def step(k, _):
            k1, k2 = jax.random.split(k)
            bm = jax.random.normal(k1, (B, NOISE), dtype=jnp.float32) * sqrt_dt
            return k2, bm

        _, bms = jax.lax.scan(step, bm_key, xs=None, length=num_t)
        return np.asarray(bms)  # [T, B, NOISE] float32


def _pack_wT(w, n_k, n_m):
    """W [out, in] -> lhsT tile layout [128, n_k, n_m*128] where
    slice [:, k, m*128:(m+1)*128] = W.T[k*128:(k+1)*128, m*128:(m+1)*128]."""
    wt = np.ascontiguousarray(w.T)  # [in, out]
    kin, mout = wt.shape
    assert kin == n_k * 128 and mout == n_m * 128
    return wt.reshape(n_k, 128, mout).transpose(1, 0, 2).reshape(128, n_k * mout)


def _build(dtype_w_name, dt_val):
    import concourse.bacc as bacc
    import concourse.mybir as mybir
    import concourse.tile as tile
    from concourse.bass import ds

    f32 = mybir.dt.float32
    dtw = {
        "f32": f32,
        "f16": mybir.dt.float16,
        "bf16": mybir.dt.bfloat16,
    }[dtype_w_name]
    AF = mybir.ActivationFunctionType
    OP = mybir.AluOpType
    AX = mybir.AxisListType

    nc = bacc.Bacc("TRN2", target_bir_lowering=False, debug=False)

    # ---------------- DRAM I/O ----------------
    d_y0 = nc.dram_tensor("y0c", [HIDDEN, BS], f32, kind="ExternalInput").ap()
    d_bm = nc.dram_tensor("bm", [1, T * BS * NOISE], f32, kind="ExternalInput").ap()
    d_wmui = nc.dram_tensor("wmui", [128, WIDTH], dtw, kind="ExternalInput").ap()
    d_wmuh = nc.dram_tensor("wmuh", [128, NHID * 2 * WIDTH], dtw, kind="ExternalInput").ap()
    d_wmuo = nc.dram_tensor("wmuo", [128, 2 * HIDDEN], dtw, kind="ExternalInput").ap()
    d_wsfi = nc.dram_tensor("wsfi", [128, WIDTH], dtw, kind="ExternalInput").ap()
    d_wsfh = nc.dram_tensor("wsfh", [128, NHID * 2 * WIDTH], dtw, kind="ExternalInput").ap()
    d_wsfo = nc.dram_tensor("wsfo", [128, 2 * HIDDEN * NOISE], dtw, kind="ExternalInput").ap()
    # interleaved per-step bias table for the two input layers:
    # cols t*4 + [mu_m0, mu_m1, sf_m0, sf_m1]
    d_tmix = nc.dram_tensor("tmix", [128, T * 4], f32, kind="ExternalInput").ap()
    d_bmuh = nc.dram_tensor("bmuh", [128, NHID * 2], f32, kind="ExternalInput").ap()
    d_bmuo = nc.dram_tensor("bmuo", [128, 1], f32, kind="ExternalInput").ap()
    d_bsfh = nc.dram_tensor("bsfh", [128, NHID * 2], f32, kind="ExternalInput").ap()
    d_bsig = nc.dram_tensor("bsig", [128, NOISE * BS], f32, kind="ExternalInput").ap()
    d_ys = nc.dram_tensor("ys", [T, HIDDEN, BS], f32, kind="ExternalOutput").ap()

    with tile.TileContext(nc) as tc:
        with (
            tc.tile_pool(name="consts", bufs=1) as consts,
            tc.tile_pool(name="yring_p", bufs=1) as yring_p,
            tc.tile_pool(name="bmbuf_p", bufs=2) as bmbuf_p,
            tc.tile_pool(name="bmrep_p", bufs=2) as bmrep_p,
            tc.tile_pool(name="zpool", bufs=2) as zpool,
            tc.tile_pool(name="hpool", bufs=3) as hpool,
            tc.tile_pool(name="gpool", bufs=3) as gpool,
            tc.tile_pool(name="spool", bufs=3) as spool,
            tc.tile_pool(name="bigpool", bufs=2) as bigpool,
            tc.tile_pool(name="ps_small", bufs=4, space="PSUM") as ps_small,
            tc.tile_pool(name="ps_sig", bufs=2, space="PSUM") as ps_sig,
        ):
            # ------------- load constants -------------
            wmui = consts.tile([128, WIDTH], dtw)
            nc.sync.dma_start(wmui[:], d_wmui)
            wmuh = consts.tile([128, NHID * 2 * WIDTH], dtw)
            nc.sync.dma_start(wmuh[:], d_wmuh)
            wmuo = consts.tile([128, 2 * HIDDEN], dtw)
            nc.sync.dma_start(wmuo[:], d_wmuo)
            wsfi = consts.tile([128, WIDTH], dtw)
            nc.sync.dma_start(wsfi[:], d_wsfi)
            wsfh = consts.tile([128, NHID * 2 * WIDTH], dtw)
            nc.sync.dma_start(wsfh[:], d_wsfh)
            wsfo = consts.tile([128, 2 * HIDDEN * NOISE], dtw)
            nc.sync.dma_start(wsfo[:], d_wsfo)
            tmix = consts.tile([128, T * 4], f32)
            nc.sync.dma_start(tmix[:], d_tmix)
            bmuh = consts.tile([128, NHID * 2], f32)
            nc.sync.dma_start(bmuh[:], d_bmuh)
            bmuo = consts.tile([128, 1], f32)
            nc.sync.dma_start(bmuo[:], d_bmuo)
            bsfh = consts.tile([128, NHID * 2], f32)
            nc.sync.dma_start(bsfh[:], d_bsfh)
            bsig = consts.tile([128, NOISE, BS], f32)
            nc.sync.dma_start(bsig[:], d_bsig.rearrange("p (n b) -> p n b", n=NOISE))

            yring = yring_p.tile([128, U, BS], f32)
            nc.sync.dma_start(yring[:, U - 1, :], d_y0)

            ITERS = T // U

            def mm(out_ap, lhsT_ap, rhs_ap, start, stop):
                nc.tensor.matmul(out_ap, lhsT_ap, rhs_ap, start=start, stop=stop)

            with tc.For_i(0, ITERS, 1, hint_engines=(mybir.EngineType.PE,)) as it:
                bmbuf = bmbuf_p.tile([1, U * BS * NOISE], f32)
                nc.sync.dma_start(
                    bmbuf[:], d_bm[0:1, ds(it * (U * BS * NOISE), U * BS * NOISE)]
                )
                for u in range(U):
                    t_expr = it * U + u
                    yprev = yring[:, (u - 1) % U, :]

                    if dtype_w_name == "f32":
                        z = yprev
                    else:
                        zt = zpool.tile([128, BS], dtw)
                        nc.vector.tensor_copy(zt[:], yprev)
                        z = zt[:]

                    # broadcast this step's bm [1, BS*NOISE] -> [128, BS*NOISE]
                    bmrep = bmrep_p.tile([128, BS, NOISE], f32)
                    nc.gpsimd.partition_broadcast(
                        bmrep[:],
                        bmbuf[0:1, u * BS * NOISE : (u + 1) * BS * NOISE],
                        channels=128,
                    )

                    # ---- input layers (t folded into bias tables) ----
                    # NOTE: dynamic (register-offset) APs on the ACT bias
                    # operand read garbage on HW, so stage this step's bias
                    # columns at a fixed address first (DVE dynamic APs work).
                    bstep = zpool.tile([128, 4], f32, tag="bstep")
                    nc.vector.tensor_copy(bstep[:], tmix[:, ds(t_expr * 4, 4)])
                    ps_mi = ps_small.tile([128, 2, BS], f32, tag="ps_small")
                    ps_si = ps_small.tile([128, 2, BS], f32, tag="ps_small")
                    for m in (0, 1):
                        mm(ps_mi[:, m, :], wmui[:, m * 128 : (m + 1) * 128], z, True, True)
                    for m in (0, 1):
                        mm(ps_si[:, m, :], wsfi[:, m * 128 : (m + 1) * 128], z, True, True)
                    h = hpool.tile([128, 2, BS], dtw, tag="h")
                    g = gpool.tile([128, 2, BS], dtw, tag="g")
                    for m in (0, 1):
                        nc.scalar.activation(
                            h[:, m, :], ps_mi[:, m, :], AF.Relu,
                            bias=bstep[:, m : m + 1],
                        )
                    for m in (0, 1):
                        nc.scalar.activation(
                            g[:, m, :], ps_si[:, m, :], AF.Silu,
                            bias=bstep[:, 2 + m : 3 + m],
                        )

                    # ---- hidden layers ----
                    for l in range(NHID):
                        ps_h = ps_small.tile([128, 2, BS], f32, tag="ps_small")
                        ps_g = ps_small.tile([128, 2, BS], f32, tag="ps_small")
                        for m in (0, 1):
                            for k in (0, 1):
                                mm(
                                    ps_h[:, m, :],
                                    wmuh[:, l * 2 * WIDTH + k * WIDTH + m * 128 : l * 2 * WIDTH + k * WIDTH + m * 128 + 128],
                                    h[:, k, :], k == 0, k == 1,
                                )
                        for m in (0, 1):
                            for k in (0, 1):
                                mm(
                                    ps_g[:, m, :],
                                    wsfh[:, l * 2 * WIDTH + k * WIDTH + m * 128 : l * 2 * WIDTH + k * WIDTH + m * 128 + 128],
                                    g[:, k, :], k == 0, k == 1,
                                )
                        hn = hpool.tile([128, 2, BS], dtw, tag="h")
                        gn = gpool.tile([128, 2, BS], dtw, tag="g")
                        for m in (0, 1):
                            nc.scalar.activation(
                                hn[:, m, :], ps_h[:, m, :], AF.Relu,
                                bias=bmuh[:, l * 2 + m : l * 2 + m + 1],
                            )
                        for m in (0, 1):
                            nc.scalar.activation(
                                gn[:, m, :], ps_g[:, m, :], AF.Silu,
                                bias=bsfh[:, l * 2 + m : l * 2 + m + 1],
                            )
                        h, g = hn, gn

                    # ---- drift output: tanh(W h + b) [128, BS] ----
                    ps_mo = ps_small.tile([128, BS], f32, tag="ps_small")
                    for k in (0, 1):
                        mm(ps_mo[:], wmuo[:, k * 128 : (k + 1) * 128], h[:, k, :], k == 0, k == 1)
                    drift = spool.tile([128, BS], f32, tag="drift")
                    nc.scalar.activation(drift[:], ps_mo[:], AF.Tanh, bias=bmuo[:, 0:1])

                    # ---- diffusion output: 32 tiles [128, BS], fixed n per tile ----
                    ps_s = ps_sig.tile([128, NOISE, BS], f32, tag="ps_sig")
                    for n in range(NOISE):
                        for k in (0, 1):
                            mm(
                                ps_s[:, n, :],
                                wsfo[:, k * HIDDEN * NOISE + n * 128 : k * HIDDEN * NOISE + n * 128 + 128],
                                g[:, k, :], k == 0, k == 1,
                            )

                    # ---- einsum: noise[h,b] = sum_n tanh(ps_s + bsig)[h,n,b] * bm[b,n]
                    usb = bigpool.tile([128, NOISE, BS], f32, tag="usb")
                    nc.vector.scalar_tensor_tensor(
                        usb[:], ps_s[:], 1.0, bsig[:], op0=OP.bypass, op1=OP.add
                    )
                    sig = bigpool.tile([128, BS, NOISE], f32, tag="sig")
                    nc.scalar.activation(
                        sig[:], usb[:].rearrange("p n b -> p b n"), AF.Tanh
                    )
                    prod = bigpool.tile([128, BS, NOISE], f32, tag="prod")
                    nc.vector.tensor_mul(prod[:], sig[:], bmrep[:])
                    noise_t = spool.tile([128, BS], f32, tag="noise")
                    nc.vector.tensor_reduce(noise_t[:], prod[:], axis=AX.X, op=OP.add)

                    # ---- y update: y' = y + drift*dt + noise ----
                    ytmp = spool.tile([128, BS], f32, tag="ytmp")
                    nc.vector.scalar_tensor_tensor(
                        ytmp[:], drift[:], float(dt_val), yprev,
                        op0=OP.mult, op1=OP.add,
                    )
                    nc.vector.tensor_add(yring[:, u, :], ytmp[:], noise_t[:])

                # write U steps to DRAM: ys[t, h, b]
                nc.sync.dma_start(
                    d_ys[ds(it * U, U), :, :].rearrange("t h b -> h t b"), yring[:]
                )

    nc.compile()
    return nc


def kernel(**inputs):
    from concourse.bass_utils import run_bass_kernel_spmd
    import ml_dtypes

    y0 = np.asarray(inputs["y0"], np.float32)
    dt = float(np.asarray(inputs["dt"]))
    t0 = float(np.asarray(inputs["t0"]))
    seed = int(np.asarray(inputs["seed"]))
    num_t = int(np.asarray(inputs["num_timesteps"]))
    assert num_t == T and y0.shape == (B, HIDDEN)

    mu_W_in = np.asarray(inputs["mu_W_in"], np.float32)   # [256, 129]
    mu_b_in = np.asarray(inputs["mu_b_in"], np.float32)   # [256]
    mu_W_h = np.asarray(inputs["mu_W_h"], np.float32)     # [3, 256, 256]
    mu_b_h = np.asarray(inputs["mu_b_h"], np.float32)     # [3, 256]
    mu_W_out = np.asarray(inputs["mu_W_out"], np.float32) # [128, 256]
    mu_b_out = np.asarray(inputs["mu_b_out"], np.float32) # [128]
    sf_W_in = np.asarray(inputs["sf_W_in"], np.float32)
    sf_b_in = np.asarray(inputs["sf_b_in"], np.float32)
    sf_W_h = np.asarray(inputs["sf_W_h"], np.float32)
    sf_b_h = np.asarray(inputs["sf_b_h"], np.float32)
    sf_W_out = np.asarray(inputs["sf_W_out"], np.float32) # [4096, 256]
    sf_b_out = np.asarray(inputs["sf_b_out"], np.float32) # [4096]

    LS = np.float32(0.909)

    np_w = {
        "f32": np.float32,
        "f16": np.float16,
        "bf16": ml_dtypes.bfloat16,
    }[DTYPE_W]

    # ---- pack weights (W^T tiles, feature-major) ----
    wmui = _pack_wT(mu_W_in[:, 1:], 1, 2).astype(np_w)           # [128, 256]
    wmuh = np.concatenate([_pack_wT(mu_W_h[l], 2, 2) for l in range(NHID)], axis=1).astype(np_w)
    wmuo = _pack_wT(mu_W_out, 2, 1).astype(np_w)                 # [128, 256]
    wsfi = _pack_wT(sf_W_in[:, 1:], 1, 2).astype(np_w)
    wsfh = np.concatenate([_pack_wT(LS * sf_W_h[l], 2, 2) for l in range(NHID)], axis=1).astype(np_w)
    # permute sf_W_out rows: r=h*NOISE+n -> r'=n*HIDDEN+h (fixed n per 128-tile)
    sfo = (LS * sf_W_out).reshape(HIDDEN, NOISE, WIDTH).transpose(1, 0, 2).reshape(HIDDEN * NOISE, WIDTH)
    wsfo = _pack_wT(sfo, 2, NOISE).astype(np_w)                  # [128, 2*4096]

    # ---- bias tables with t folded in ----
    tvals = np.float32(t0) + np.arange(T, dtype=np.float32) * np.float32(dt)
    tbl_mu = mu_b_in[None, :] + tvals[:, None] * mu_W_in[:, 0][None, :]  # [T, 256]
    tbl_sf = sf_b_in[None, :] + tvals[:, None] * sf_W_in[:, 0][None, :]
    tmix = np.ascontiguousarray(
        np.concatenate([tbl_mu.reshape(T, 2, 128), tbl_sf.reshape(T, 2, 128)], axis=1)
        .transpose(2, 0, 1)
        .reshape(128, T * 4),
        np.float32,
    )
    bmuh = np.ascontiguousarray(
        mu_b_h.reshape(NHID * 2, 128).T.reshape(128, NHID * 2), np.float32
    )
    bsfh = np.ascontiguousarray(
        sf_b_h.reshape(NHID * 2, 128).T.reshape(128, NHID * 2), np.float32
    )
    bmuo = np.ascontiguousarray(mu_b_out[:, None], np.float32)   # [128, 1]
    # sig bias: [h, n] replicated over b is NOT needed; layout (n, b): value b_out[h*NOISE+n]
    bsig_hn = sf_b_out.reshape(HIDDEN, NOISE)                    # [h, n]
    bsig = np.ascontiguousarray(
        np.repeat(bsig_hn[:, :, None], BS, axis=2).reshape(128, NOISE * BS), np.float32
    )

    # ---- Brownian increments, bit-matching reference PRNG ----
    bm = _gen_bm(seed, T, dt)  # [T, B, NOISE]

    key = (DTYPE_W,)
    if key not in _CACHE:
        _CACHE[key] = _build(DTYPE_W, dt)
    nc = _CACHE[key]

    shared = dict(
        wmui=wmui, wmuh=wmuh, wmuo=wmuo, wsfi=wsfi, wsfh=wsfh, wsfo=wsfo,
        tmix=tmix, bmuh=bmuh, bmuo=bmuo, bsfh=bsfh, bsig=bsig,
    )
    in_maps = []
    for c in range(NCORES):
        sl = slice(c * BS, (c + 1) * BS)
        in_maps.append(
            dict(
                shared,
                y0c=np.ascontiguousarray(y0[sl].T, np.float32),          # [128, BS]
                bm=np.ascontiguousarray(bm[:, sl, :].reshape(1, -1), np.float32),
            )
        )

    res = run_bass_kernel_spmd(nc, in_maps, core_ids=list(range(NCORES)))

    ys = np.empty((T, B, HIDDEN), np.float32)
    for c in range(NCORES):
        ys[:, c * BS : (c + 1) * BS, :] = res.results[c]["ys"].transpose(0, 2, 1)
    return ys


# revision 10
# speedup vs baseline: 3.4376x; 3.4376x over previous
"""Trainium2 Bass kernel: Neural SDE (Euler-Maruyama scan, drift+diffusion MLPs).

Strategy:
  - Data-parallel across 8 NeuronCores over the trajectory batch (256 -> 32/core).
  - Inside each core: 2048 sequential timesteps in a For_i dynamic loop
    (U steps unrolled per iteration).
  - Activations kept feature-major [feat, batch] on SBUF partitions; all
    matmuls are weight-stationary: out[f_out, b] = (W^T)tile.T @ H.
  - Brownian increments are precomputed on host with JAX CPU, bit-matching
    the reference PRNG key chain, and streamed in via DMA.
  - The scalar time input t is folded into a per-step bias table for the
    first layer of both MLPs (z = [t, y] -> W[:,0]*t + b becomes bias(t)).
  - lipswish(x) = 0.909*silu(x): the 0.909 is folded into the next layer's
    weights; the HW Silu activation is used directly.
  - The diffusion einsum sum_n sig[b,h,n]*bm[b,n] runs on the vector engine
    with the noise weight rows permuted so each 128-row tile has a fixed n.
"""

import sys

if "/opt/trn_rl_repo" not in sys.path:
    sys.path.insert(0, "/opt/trn_rl_repo")

import numpy as np

B, HIDDEN, NOISE, WIDTH = 256, 128, 32, 256
NHID = 3
T = 2048
NCORES = 8
BS = B // NCORES  # 32 per-core batch
U = 8             # timesteps per For_i iteration

# weight/hidden-activation dtype: "f32", "f16" (half) or "bf16"
import os as _os
DTYPE_W = _os.environ.get("NSDE_DTYPE", "f16")

_CACHE = {}


def _gen_bm(seed, num_t, dt):
    """Replicate reference's jax.random key chain for Brownian increments."""
    import jax
    import jax.numpy as jnp

    cpu = jax.devices("cpu")[0]
    with jax.default_device(cpu):
        key = jax.random.key(int(seed))
        _, bm_key = jax.random.split(key)
        sqrt_dt = jnp.sqrt(jnp.float32(dt))

        def step(k, _):
            k1, k2 = jax.random.split(k)
            bm = jax.random.normal(k1, (B, NOISE), dtype=jnp.float32) * sqrt_dt
            return k2, bm

        _, bms = jax.lax.scan(step, bm_key, xs=None, length=num_t)
        return np.asarray(bms)  # [T, B, NOISE] float32


def _pack_wT(w, n_k, n_m):
    """W [out, in] -> lhsT tile layout [128, n_k, n_m*128] where
    slice [:, k, m*128:(m+1)*128] = W.T[k*128:(k+1)*128, m*128:(m+1)*128]."""
    wt = np.ascontiguousarray(w.T)  # [in, out]
    kin, mout = wt.shape
    assert kin == n_k * 128 and mout == n_m * 128
    return wt.reshape(n_k, 128, mout).transpose(1, 0, 2).reshape(128, n_k * mout)


def _build(dtype_w_name, dt_val):
    import concourse.bacc as bacc
    import concourse.mybir as mybir
    import concourse.tile as tile
    from concourse.bass import ds

    f32 = mybir.dt.float32
    dtw = {
        "f32": f32,
        "f16": mybir.dt.float16,
        "bf16": mybir.dt.bfloat16,
    }[dtype_w_name]
    AF = mybir.ActivationFunctionType
    OP = mybir.AluOpType
    AX = mybir.AxisListType

    nc = bacc.Bacc("TRN2", target_bir_lowering=False, debug=False)

    # ---------------- DRAM I/O ----------------
    d_y0 = nc.dram_tensor("y0c", [HIDDEN, BS], f32, kind="ExternalInput").ap()
    d_bm = nc.dram_tensor("bm", [1, T * BS * NOISE], f32, kind="ExternalInput").ap()
    d_wmui = nc.dram_tensor("wmui", [128, WIDTH], dtw, kind="ExternalInput").ap()
    d_wmuh = nc.dram_tensor("wmuh", [128, NHID * 2 * WIDTH], dtw, kind="ExternalInput").ap()
    d_wmuo = nc.dram_tensor("wmuo", [128, 2 * HIDDEN], dtw, kind="ExternalInput").ap()
    d_wsfi = nc.dram_tensor("wsfi", [128, WIDTH], dtw, kind="ExternalInput").ap()
    d_wsfh = nc.dram_tensor("wsfh", [128, NHID * 2 * WIDTH], dtw, kind="ExternalInput").ap()
    d_wsfo = nc.dram_tensor("wsfo", [128, 2 * HIDDEN * NOISE], dtw, kind="ExternalInput").ap()
    # interleaved per-step bias table for the two input layers:
    # cols t*4 + [mu_m0, mu_m1, sf_m0, sf_m1]
    d_tmix = nc.dram_tensor("tmix", [128, T * 4], f32, kind="ExternalInput").ap()
    d_bmuh = nc.dram_tensor("bmuh", [128, NHID * 2], f32, kind="ExternalInput").ap()
    d_bmuo = nc.dram_tensor("bmuo", [128, 1], f32, kind="ExternalInput").ap()
    d_bsfh = nc.dram_tensor("bsfh", [128, NHID * 2], f32, kind="ExternalInput").ap()
    d_bsig = nc.dram_tensor("bsig", [128, NOISE * BS], f32, kind="ExternalInput").ap()
    d_ys = nc.dram_tensor("ys", [T, HIDDEN, BS], f32, kind="ExternalOutput").ap()

    with tile.TileContext(nc) as tc:
        with (
            tc.tile_pool(name="consts", bufs=1) as consts,
            tc.tile_pool(name="yring_p", bufs=1) as yring_p,
            tc.tile_pool(name="bmbuf_p", bufs=2) as bmbuf_p,
            tc.tile_pool(name="bmrep_p", bufs=2) as bmrep_p,
            tc.tile_pool(name="zpool", bufs=2) as zpool,
            tc.tile_pool(name="hpool", bufs=3) as hpool,
            tc.tile_pool(name="gpool", bufs=3) as gpool,
            tc.tile_pool(name="spool", bufs=3) as spool,
            tc.tile_pool(name="bigpool", bufs=2) as bigpool,
            tc.tile_pool(name="ps_small", bufs=4, space="PSUM") as ps_small,
            tc.tile_pool(name="ps_sig", bufs=2, space="PSUM") as ps_sig,
        ):
            # ------------- load constants -------------
            wmui = consts.tile([128, WIDTH], dtw)
            nc.sync.dma_start(wmui[:], d_wmui)
            wmuh = consts.tile([128, NHID * 2 * WIDTH], dtw)
            nc.sync.dma_start(wmuh[:], d_wmuh)
            wmuo = consts.tile([128, 2 * HIDDEN], dtw)
            nc.sync.dma_start(wmuo[:], d_wmuo)
            wsfi = consts.tile([128, WIDTH], dtw)
            nc.sync.dma_start(wsfi[:], d_wsfi)
            wsfh = consts.tile([128, NHID * 2 * WIDTH], dtw)
            nc.sync.dma_start(wsfh[:], d_wsfh)
            wsfo = consts.tile([128, 2 * HIDDEN * NOISE], dtw)
            nc.sync.dma_start(wsfo[:], d_wsfo)
            tmix = consts.tile([128, T * 4], f32)
            nc.sync.dma_start(tmix[:], d_tmix)
            bmuh = consts.tile([128, NHID * 2], f32)
            nc.sync.dma_start(bmuh[:], d_bmuh)
            bmuo = consts.tile([128, 1], f32)
            nc.sync.dma_start(bmuo[:], d_bmuo)
            bsfh = consts.tile([128, NHID * 2], f32)
            nc.sync.dma_start(bsfh[:], d_bsfh)
            bsig = consts.tile([128, NOISE, BS], f32)
            nc.sync.dma_start(bsig[:], d_bsig.rearrange("p (n b) -> p n b", n=NOISE))

            yring = yring_p.tile([128, U, BS], f32)
            nc.sync.dma_start(yring[:, U - 1, :], d_y0)

            ITERS = T // U

            def mm(out_ap, lhsT_ap, rhs_ap, start, stop):
                nc.tensor.matmul(out_ap, lhsT_ap, rhs_ap, start=start, stop=stop)

            with tc.For_i(0, ITERS, 1, hint_engines=(mybir.EngineType.PE,)) as it:
                bmbuf = bmbuf_p.tile([1, U * BS * NOISE], f32)
                nc.sync.dma_start(
                    bmbuf[:], d_bm[0:1, ds(it * (U * BS * NOISE), U * BS * NOISE)]
                )
                for u in range(U):
                    t_expr = it * U + u
                    yprev = yring[:, (u - 1) % U, :]

                    if dtype_w_name == "f32":
                        z = yprev
                    else:
                        zt = zpool.tile([128, BS], dtw)
                        nc.vector.tensor_copy(zt[:], yprev)
                        z = zt[:]

                    # broadcast this step's bm [1, BS*NOISE] -> [128, BS*NOISE]
                    bmrep = bmrep_p.tile([128, BS, NOISE], f32)
                    nc.gpsimd.partition_broadcast(
                        bmrep[:],
                        bmbuf[0:1, u * BS * NOISE : (u + 1) * BS * NOISE],
                        channels=128,
                    )

                    # ---- input layers (t folded into bias tables) ----
                    # NOTE: dynamic (register-offset) APs on the ACT bias
                    # operand read garbage on HW, so stage this step's bias
                    # columns at a fixed address first (DVE dynamic APs work).
                    bstep = zpool.tile([128, 4], f32, tag="bstep")
                    nc.vector.tensor_copy(bstep[:], tmix[:, ds(t_expr * 4, 4)])
                    ps_mi = ps_small.tile([128, 2, BS], f32, tag="ps_small")
                    ps_si = ps_small.tile([128, 2, BS], f32, tag="ps_small")
                    for m in (0, 1):
                        mm(ps_mi[:, m, :], wmui[:, m * 128 : (m + 1) * 128], z, True, True)
                    for m in (0, 1):
                        mm(ps_si[:, m, :], wsfi[:, m * 128 : (m + 1) * 128], z, True, True)
                    h = hpool.tile([128, 2, BS], dtw, tag="h")
                    g = gpool.tile([128, 2, BS], dtw, tag="g")
                    for m in (0, 1):
                        nc.scalar.activation(
                            h[:, m, :], ps_mi[:, m, :], AF.Relu,
                            bias=bstep[:, m : m + 1],
                        )
                    for m in (0, 1):
                        nc.scalar.activation(
                            g[:, m, :], ps_si[:, m, :], AF.Silu,
                            bias=bstep[:, 2 + m : 3 + m],
                        )

                    # ---- hidden layers ----
                    for l in range(NHID):
                        ps_h = ps_small.tile([128, 2, BS], f32, tag="ps_small")
                        ps_g = ps_small.tile([128, 2, BS], f32, tag="ps_small")
                        for m in (0, 1):
                            for k in (0, 1):
                                mm(
                                    ps_h[:, m, :],
                                    wmuh[:, l * 2 * WIDTH + k * WIDTH + m * 128 : l * 2 * WIDTH + k * WIDTH + m * 128 + 128],
                                    h[:, k, :], k == 0, k == 1,
                                )
                        for m in (0, 1):
                            for k in (0, 1):
                                mm(
                                    ps_g[:, m, :],
                                    wsfh[:, l * 2 * WIDTH + k * WIDTH + m * 128 : l * 2 * WIDTH + k * WIDTH + m * 128 + 128],
                                    g[:, k, :], k == 0, k == 1,
                                )
                        hn = hpool.tile([128, 2, BS], dtw, tag="h")
                        gn = gpool.tile([128, 2, BS], dtw, tag="g")
                        for m in (0, 1):
                            nc.scalar.activation(
                                hn[:, m, :], ps_h[:, m, :], AF.Relu,
                                bias=bmuh[:, l * 2 + m : l * 2 + m + 1],
                            )
                        for m in (0, 1):
                            nc.scalar.activation(
                                gn[:, m, :], ps_g[:, m, :], AF.Silu,
                                bias=bsfh[:, l * 2 + m : l * 2 + m + 1],
                            )
                        h, g = hn, gn

                    # ---- drift output: tanh(W h + b) [128, BS] ----
                    ps_mo = ps_small.tile([128, BS], f32, tag="ps_small")
                    for k in (0, 1):
                        mm(ps_mo[:], wmuo[:, k * 128 : (k + 1) * 128], h[:, k, :], k == 0, k == 1)
                    drift = spool.tile([128, BS], f32, tag="drift")
                    nc.scalar.activation(drift[:], ps_mo[:], AF.Tanh, bias=bmuo[:, 0:1])

                    # ---- diffusion output: 32 tiles [128, BS], fixed n per tile ----
                    ps_s = ps_sig.tile([128, NOISE, BS], f32, tag="ps_sig")
                    for n in range(NOISE):
                        for k in (0, 1):
                            mm(
                                ps_s[:, n, :],
                                wsfo[:, k * HIDDEN * NOISE + n * 128 : k * HIDDEN * NOISE + n * 128 + 128],
                                g[:, k, :], k == 0, k == 1,
                            )

                    # ---- einsum: noise[h,b] = sum_n tanh(ps_s + bsig)[h,n,b] * bm[b,n]
                    usb = bigpool.tile([128, NOISE, BS], f32, tag="usb")
                    nc.vector.scalar_tensor_tensor(
                        usb[:], ps_s[:], 1.0, bsig[:], op0=OP.bypass, op1=OP.add
                    )
                    sig = bigpool.tile([128, BS, NOISE], f32, tag="sig")
                    nc.scalar.activation(
                        sig[:], usb[:].rearrange("p n b -> p b n"), AF.Tanh
                    )
                    prod = bigpool.tile([128, BS, NOISE], f32, tag="prod")
                    nc.vector.tensor_mul(prod[:], sig[:], bmrep[:])
                    noise_t = spool.tile([128, BS], f32, tag="noise")
                    nc.vector.tensor_reduce(noise_t[:], prod[:], axis=AX.X, op=OP.add)

                    # ---- y update: y' = y + drift*dt + noise ----
                    ytmp = spool.tile([128, BS], f32, tag="ytmp")
                    nc.vector.scalar_tensor_tensor(
                        ytmp[:], drift[:], float(dt_val), yprev,
                        op0=OP.mult, op1=OP.add,
                    )
                    nc.vector.tensor_add(yring[:, u, :], ytmp[:], noise_t[:])

                # write U steps to DRAM: ys[t, h, b]
                nc.sync.dma_start(
                    d_ys[ds(it * U, U), :, :].rearrange("t h b -> h t b"), yring[:]
                )

    nc.compile()
    return nc


def kernel(**inputs):
    from concourse.bass_utils import run_bass_kernel_spmd
    import ml_dtypes

    y0 = np.asarray(inputs["y0"], np.float32)
    dt = float(np.asarray(inputs["dt"]))
    t0 = float(np.asarray(inputs["t0"]))
    seed = int(np.asarray(inputs["seed"]))
    num_t = int(np.asarray(inputs["num_timesteps"]))
    assert num_t == T and y0.shape == (B, HIDDEN)

    mu_W_in = np.asarray(inputs["mu_W_in"], np.float32)   # [256, 129]
    mu_b_in = np.asarray(inputs["mu_b_in"], np.float32)   # [256]
    mu_W_h = np.asarray(inputs["mu_W_h"], np.float32)     # [3, 256, 256]
    mu_b_h = np.asarray(inputs["mu_b_h"], np.float32)     # [3, 256]
    mu_W_out = np.asarray(inputs["mu_W_out"], np.float32) # [128, 256]
    mu_b_out = np.asarray(inputs["mu_b_out"], np.float32) # [128]
    sf_W_in = np.asarray(inputs["sf_W_in"], np.float32)
    sf_b_in = np.asarray(inputs["sf_b_in"], np.float32)
    sf_W_h = np.asarray(inputs["sf_W_h"], np.float32)
    sf_b_h = np.asarray(inputs["sf_b_h"], np.float32)
    sf_W_out = np.asarray(inputs["sf_W_out"], np.float32) # [4096, 256]
    sf_b_out = np.asarray(inputs["sf_b_out"], np.float32) # [4096]

    LS = np.float32(0.909)

    np_w = {
        "f32": np.float32,
        "f16": np.float16,
        "bf16": ml_dtypes.bfloat16,
    }[DTYPE_W]

    # ---- pack weights (W^T tiles, feature-major) ----
    wmui = _pack_wT(mu_W_in[:, 1:], 1, 2).astype(np_w)           # [128, 256]
    wmuh = np.concatenate([_pack_wT(mu_W_h[l], 2, 2) for l in range(NHID)], axis=1).astype(np_w)
    wmuo = _pack_wT(mu_W_out, 2, 1).astype(np_w)                 # [128, 256]
    wsfi = _pack_wT(sf_W_in[:, 1:], 1, 2).astype(np_w)
    wsfh = np.concatenate([_pack_wT(LS * sf_W_h[l], 2, 2) for l in range(NHID)], axis=1).astype(np_w)
    # permute sf_W_out rows: r=h*NOISE+n -> r'=n*HIDDEN+h (fixed n per 128-tile)
    sfo = (LS * sf_W_out).reshape(HIDDEN, NOISE, WIDTH).transpose(1, 0, 2).reshape(HIDDEN * NOISE, WIDTH)
    wsfo = _pack_wT(sfo, 2, NOISE).astype(np_w)                  # [128, 2*4096]

    # ---- bias tables with t folded in ----
    tvals = np.float32(t0) + np.arange(T, dtype=np.float32) * np.float32(dt)
    tbl_mu = mu_b_in[None, :] + tvals[:, None] * mu_W_in[:, 0][None, :]  # [T, 256]
    tbl_sf = sf_b_in[None, :] + tvals[:, None] * sf_W_in[:, 0][None, :]
    tmix = np.ascontiguousarray(
        np.concatenate([tbl_mu.reshape(T, 2, 128), tbl_sf.reshape(T, 2, 128)], axis=1)
        .transpose(2, 0, 1)
        .reshape(128, T * 4),
        np.float32,
    )
    bmuh = np.ascontiguousarray(
        mu_b_h.reshape(NHID * 2, 128).T.reshape(128, NHID * 2), np.float32
    )
    bsfh = np.ascontiguousarray(
        sf_b_h.reshape(NHID * 2, 128).T.reshape(128, NHID * 2), np.float32
    )
    bmuo = np.ascontiguousarray(mu_b_out[:, None], np.float32)   # [128, 1]
    # sig bias: [h, n] replicated over b is NOT needed; layout (n, b): value b_out[h*NOISE+n]
    bsig_hn = sf_b_out.reshape(HIDDEN, NOISE)                    # [h, n]
    bsig = np.ascontiguousarray(
        np.repeat(bsig_hn[:, :, None], BS, axis=2).reshape(128, NOISE * BS), np.float32
    )

    # ---- Brownian increments, bit-matching reference PRNG ----
    bm = _gen_bm(seed, T, dt)  # [T, B, NOISE]

    key = (DTYPE_W,)
    if key not in _CACHE:
        _CACHE[key] = _build(DTYPE_W, dt)
    nc = _CACHE[key]

    shared = dict(
        wmui=wmui, wmuh=wmuh, wmuo=wmuo, wsfi=wsfi, wsfh=wsfh, wsfo=wsfo,
        tmix=tmix, bmuh=bmuh, bmuo=bmuo, bsfh=bsfh, bsig=bsig,
    )
    in_maps = []
    for c in range(NCORES):
        sl = slice(c * BS, (c + 1) * BS)
        in_maps.append(
            dict(
                shared,
                y0c=np.ascontiguousarray(y0[sl].T, np.float32),          # [128, BS]
                bm=np.ascontiguousarray(bm[:, sl, :].reshape(1, -1), np.float32),
            )
        )

    res = run_bass_kernel_spmd(nc, in_maps, core_ids=list(range(NCORES)))

    ys = np.empty((T, B, HIDDEN), np.float32)
    for c in range(NCORES):
        ys[:, c * BS : (c + 1) * BS, :] = res.results[c]["ys"].transpose(0, 2, 1)
    return ys
